# revision 16
# baseline (speedup 1.0000x reference)
"""Multi-head attention (B=4, S=2048, D=1024, H=16, dk=dv=64) on 8 TRN2 NeuronCores.

Sharding: batch x head-half. Core c handles batch b = c//2 and heads
hh*8..hh*8+8 where hh = c%2. Each core computes its 8 heads' attention plus
the partial output projection (row-parallel fc); the host sums the two
partials per batch and adds the output bias.

Device algorithm per core (all matmul inputs bf16, PSUM accumulation fp32):
  - xT = x.T on host (D on partitions); every matmul contracts over the
    partition dimension with zero on-device transposes.
  - Q^T, K^T = W X^T laid out [feat, seq]; the 128-row feature chunk fc
    holds the head PAIR (2fc, 2fc+1): even head's dk on partitions 0-63,
    odd head's on 64-127.  KT2 uses the identical layout (no per-head
    zero-padded stripes).
  - Scores: per (pair, kpos-chunk, q-window) TWO concurrent row-tiled
    K=64 matmuls (tile_position (0,0)/(64,0) auto-derived from the
    partition halves) write [S^T_even | S^T_odd] into one 2-bank PSUM
    tile; ONE exp activation (ScalarE, the attention-phase bottleneck)
    covers both heads.  This halves score PE time vs K=128 zero-padded
    matmuls (row-tile concurrency measured 1.71x on HW).
  - 64-row-mode score blocks are batched (2 kc per block) and alternate
    with 128-mode batches (PV + projection slices) because PE tiling-mode
    switches drain the array (~110ns each, HW-measured).
  - V laid [kpos, 65] per head: col 64 = ones so the softmax denominator
    rides the PV matmul.  PV: C^T_aug accumulated over kpos per head.
  - normalize: r = 1/l via DRAM-bounce partition broadcast (off critical
    path); odd head's CT rows written via SBUF->SBUF DMA (DVE lanes
    cannot partition-shift).
  - out_partial = CT.T @ Wo_c^T per 128-qpos chunk, fp32 to DRAM.
  - Work is interleaved via a FIFO filler queue: q-windows of 512 qpos;
    per (pair, 2kc) score block the queue supplies ~1.7us of 128-mode
    work (PV catch-up, K/Q/V projection passes, output projection of the
    previous window) so the PE never idles and ScalarE stays saturated.
"""

import sys

if "/opt/trn_rl_repo" not in sys.path:
    sys.path.insert(0, "/opt/trn_rl_repo")

from collections import deque
from contextlib import ExitStack

import ml_dtypes
import numpy as np

import concourse.bass as bass
import concourse.tile as tile
from concourse import bacc, mybir
from concourse.bass_utils import run_bass_kernel_spmd

BF16 = mybir.dt.bfloat16
F32 = mybir.dt.float32
P = 128

B, S, D = 4, 2048, 1024
H, DH = 16, 64
G = 512          # head-group width per core: 8 heads x 64
NH = G // DH     # 8 heads per core
NP = NH // 2     # 4 head pairs per core
DC = D // P      # contraction chunks over model dim
KC = S // P      # kpos chunks of 128
NW = S // 512    # q windows of 512
SCALE = 1.0 / 8.0  # 1/sqrt(dk)
ES_BUFS = 16     # exp-score SBUF ring (decouples ScalarE from PV catch-up)


def _emit(ctx, tc, io):
    nc = tc.nc
    EXP = mybir.ActivationFunctionType.Exp

    wpool = ctx.enter_context(tc.tile_pool(name="w", bufs=1))
    perm = ctx.enter_context(tc.tile_pool(name="perm", bufs=1))
    xpool = ctx.enter_context(tc.tile_pool(name="x", bufs=2))
    epool = ctx.enter_context(tc.tile_pool(name="e", bufs=ES_BUFS))
    small = ctx.enter_context(tc.tile_pool(name="small", bufs=2))
    opool = ctx.enter_context(tc.tile_pool(name="o", bufs=2))
    dpool = ctx.enter_context(tc.tile_pool(name="d", bufs=3, space="DRAM"))
    ps_s = ctx.enter_context(tc.tile_pool(name="pss", bufs=2, space="PSUM"))
    ps_pv = ctx.enter_context(tc.tile_pool(name="psv", bufs=2, space="PSUM"))
    ps_a = ctx.enter_context(tc.tile_pool(name="psa", bufs=2, space="PSUM"))

    # --- persistent weights / biases / activations ---
    wq_sb = wpool.tile([P, DC, G], BF16, name="wq_sb")
    wk_sb = wpool.tile([P, DC, G], BF16, name="wk_sb")
    wv_sb = wpool.tile([P, DC, G], BF16, name="wv_sb")
    wo_sb = wpool.tile([P, NP, D], BF16, name="wo_sb")
    bq_sb = wpool.tile([P, NP], F32, name="bq_sb")
    bk_sb = wpool.tile([P, NP], F32, name="bk_sb")
    bv_sb = wpool.tile([P, G], F32, name="bv_sb")
    bv4 = bv_sb.rearrange("p (h c) -> p h c", h=NH)

    QT = perm.tile([P, NP, S], BF16, name="QT")
    KT2 = perm.tile([P, NP, S], BF16, name="KT2")
    V4 = perm.tile([P, KC, NH, DH + 1], BF16, name="V4")
    CT = perm.tile([P, NP, S], BF16, name="CT")
    nc.vector.memset(V4[:, :, :, DH:DH + 1], 1.0)

    # --- input x tiles + DMA rings ---
    # scalar ring: wk + K chunks 1-3; sync ring: xk0/xq0 + biases + V chunks
    # (+ later out/normalize traffic); gpsimd (SWDGE) ring: wq/wv/wo.
    xk = {}
    for qc in range(4):
        xk[qc] = xpool.tile([P, DC, 512], BF16, name=f"xk{qc}", tag="xk",
                            bufs=4)
    xq, xv = {}, {}
    xq[0] = xpool.tile([P, DC, 512], BF16, name="xq0", tag="xq", bufs=2)
    xv[0] = xpool.tile([P, DC, 512], BF16, name="xv0", tag="xv", bufs=1)

    def xsrc(key, qc):
        return io[key][qc].rearrange("p (dc s) -> p dc s", s=512)

    nc.scalar.dma_start(wk_sb[:], io["wkT"][:])
    nc.sync.dma_start(bk_sb[:], io["bkc"].rearrange("(fc p) -> p fc", p=P))
    nc.sync.dma_start(xk[0][:], xsrc("xkT", 0))
    nc.gpsimd.dma_start(wq_sb[:], io["wqT"][:])
    nc.sync.dma_start(xq[0][:], xsrc("xqT", 0))
    nc.sync.dma_start(bq_sb[:], io["bqc"].rearrange("(fc p) -> p fc", p=P))
    nc.sync.dma_start(bv_sb[:],
                      io["bvc"].unsqueeze(0).partition_broadcast(P))
    for qc in range(1, 4):
        nc.scalar.dma_start(xk[qc][:], xsrc("xkT", qc))
    nc.gpsimd.dma_start(wv_sb[:], io["wvT"][:])
    nc.sync.dma_start(xv[0][:], xsrc("xvT", 0))
    nc.gpsimd.dma_start(wo_sb[:], io["woT"][:])

    # --- projection passes (128-mode PE work) ---
    def kq_pass(tname, qc, fc):
        if tname == "k":
            xt, wsb, bsb, dest = xk[qc], wk_sb, bk_sb, KT2
        else:
            xt, wsb, bsb, dest = xq[qc], wq_sb, bq_sb, QT
        ps = ps_a.tile([P, 512], F32, name=f"p{tname}{qc}f{fc}", tag="acc")
        for dc in range(DC):
            nc.tensor.matmul(ps[:], wsb[:, dc, fc * P:(fc + 1) * P],
                             xt[:, dc, :], start=(dc == 0), stop=(dc == DC - 1))
        nc.vector.tensor_scalar_add(
            out=dest[:, fc, qc * 512:(qc + 1) * 512], in0=ps[:],
            scalar1=bsb[:, fc:fc + 1])

    def v_pass(qc, s4):
        kc = qc * 4 + s4
        ps = ps_a.tile([P, 512], F32, name=f"pv{kc}", tag="acc")
        for dc in range(DC):
            nc.tensor.matmul(ps[:], xv[qc][:, dc, s4 * P:(s4 + 1) * P],
                             wv_sb[:, dc, :], start=(dc == 0),
                             stop=(dc == DC - 1))
        nc.vector.tensor_add(out=V4[:, kc, :, 0:DH],
                             in0=ps.rearrange("p (h c) -> p h c", h=NH),
                             in1=bv4)

    def outproj(w, sc, oc):
        col = (w * 4 + sc) * P
        ops = ps_a.tile([P, 512], F32, name=f"op{w}s{sc}o{oc}", tag="acc")
        for fc in range(NP):
            nc.tensor.matmul(ops[:], CT[:, fc, col:col + P],
                             wo_sb[:, fc, oc * 512:(oc + 1) * 512],
                             start=(fc == 0), stop=(fc == NP - 1))
        osb = opool.tile([P, 512], F32, name=f"ob{w}s{sc}o{oc}", tag="ob")
        nc.vector.tensor_copy(out=osb[:], in_=ops[:])
        nc.sync.dma_start(io["out"][col:col + P, oc * 512:(oc + 1) * 512],
                          osb[:])

    # --- attention pieces ---
    est = {}   # (w, fc, kc) -> es tile awaiting PV
    cps = {}   # (w, fc) -> (cpsE, cpsO) accumulators
    nv = [0]         # V chunks emitted (gates w0 PV)
    es_out = [0]     # es tiles emitted minus consumed (ring occupancy)
    norm_done = [0]  # normalize closures emitted (frees ps_pv pairs)
    pv_done = {}     # pair_seq -> PV closures emitted (8 completes a pair)

    def make_pv(w, fc, k0):
        pair = w * NP + fc

        def ready():
            # strictly in-order within the pair (accumulation group!)
            if pv_done.get(pair, 0) != k0 // 2:
                return False
            # w0: V chunks k0, k0+1 must be projected first
            if w == 0 and nv[0] < k0 + 2:
                return False
            # first PV of a pair claims both ps_pv slots -> the previous
            # pair must have been normalized (slots freed) beforehand
            if k0 == 0 and pair > 0 and norm_done[0] < pair:
                return False
            return True

        def f():
            if (w, fc) not in cps:
                cps[(w, fc)] = (
                    ps_pv.tile([P, 512], F32, name=f"cE{w}f{fc}", tag="pv"),
                    ps_pv.tile([P, 512], F32, name=f"cO{w}f{fc}", tag="pv"))
            cE, cO = cps[(w, fc)]
            for kc in (k0, k0 + 1):
                es = est.pop((w, fc, kc))
                nc.tensor.matmul(cE[0:DH + 1, :], V4[:, kc, 2 * fc, :],
                                 es[:, 0:512], start=(kc == 0),
                                 stop=(kc == KC - 1))
                nc.tensor.matmul(cO[0:DH + 1, :], V4[:, kc, 2 * fc + 1, :],
                                 es[:, 512:1024], start=(kc == 0),
                                 stop=(kc == KC - 1))
            es_out[0] -= 2
            pv_done[pair] = pv_done.get(pair, 0) + 1
        return (1100, ready, f)

    def make_normalize(w, fc):
        pair = w * NP + fc

        def ready():
            return pv_done.get(pair, 0) == KC // 2

        def f():
            norm_done[0] += 1
            cE, cO = cps.pop((w, fc))
            q0 = w * 512
            for par, c in ((0, cE), (1, cO)):
                l1 = small.tile([P, 512], BF16, name=f"l{w}f{fc}p{par}",
                                tag="l1")
                nc.vector.tensor_copy(out=l1[DH:DH + 1, :],
                                      in_=c[DH:DH + 1, :])
                csb = small.tile([DH, 512], F32, name=f"cs{w}f{fc}p{par}",
                                 tag="csb")
                nc.vector.tensor_copy(out=csb[:], in_=c[0:DH, :])
                rd = dpool.tile([1, 512], BF16, name=f"rd{w}f{fc}p{par}",
                                tag="rd")
                nc.sync.dma_start(rd[:], l1[DH:DH + 1, :])
                lbb = small.tile([DH, 512], F32, name=f"lb{w}f{fc}p{par}",
                                 tag="lbb", bufs=1)
                nc.gpsimd.dma_start(lbb[:], rd[0].partition_broadcast(DH))
                rbb = small.tile([DH, 512], F32, name=f"rb{w}f{fc}p{par}",
                                 tag="rbb", bufs=1)
                nc.vector.reciprocal_approx_fast(rbb[:], lbb[:])
                if par == 0:
                    nc.vector.tensor_mul(out=CT[0:DH, fc, q0:q0 + 512],
                                         in0=csb[:], in1=rbb[:])
                else:
                    tmp = small.tile([DH, 512], BF16, name=f"t{w}f{fc}",
                                     tag="tmp", bufs=1)
                    nc.vector.tensor_mul(out=tmp[:], in0=csb[:], in1=rbb[:])
                    nc.sync.dma_start(CT[DH:2 * DH, fc, q0:q0 + 512], tmp[:])
        return (300, ready, f)

    def make_outproj(w, sc, oc):
        def ready():
            return norm_done[0] >= NP * (w + 1)   # window w fully normalized
        return (900, ready, lambda: outproj(w, sc, oc))

    # --- work queues drained between score blocks ---
    # urgent: next window's Q passes (tiny, gate the window start)
    # pvq:    predicate-gated ordered work (PV, normalize, outproj) --
    #         emission order must make every runtime FIFO dependency point
    #         backwards, so ring-slot consumers are emitted before claimants
    # fillers: K/V projection passes (dep-safe at any point)
    urgent = deque()
    pvq = deque()
    fillers = deque()

    def pop_pvq():
        for i, (cost, ready, fn) in enumerate(pvq):
            if ready():
                del pvq[i]
                fn()
                return cost
        return 0

    def pop_filler(only_v=False):
        for i, (cost, fn, is_v) in enumerate(fillers):
            if is_v or not only_v:
                del fillers[i]
                fn()
                return cost
        return 0

    def drain(budget):
        spent = 0
        while urgent and spent < budget:
            cost, fn = urgent.popleft()
            fn()
            spent += cost
        toggle = True
        while spent < budget:
            c = pop_pvq() if toggle else pop_filler()
            if c == 0:
                c = pop_filler() if toggle else pop_pvq()
            if c == 0:
                break
            spent += c
            toggle = not toggle

    def ensure_es(need):
        # keep the es ring's future slot-consumers ahead of new claimants
        guard = 0
        while es_out[0] + need > ES_BUFS:
            c = pop_pvq() or pop_filler(only_v=True) or pop_filler()
            assert c, "emission wedged: es ring has no ready consumer"
            guard += 1
            assert guard < 10000

    def kf(qc, fc):
        return (1700, lambda: kq_pass("k", qc, fc), False)

    def qf(w, fc):
        return (1700, lambda: kq_pass("q", w, fc), False)

    def vf(qc, s4):
        def f():
            v_pass(qc, s4)
            nv[0] += 1
        return (1700, f, True)

    # lead-in: unblock pair-0/window-0 scores with two projection passes
    kq_pass("k", 0, 0)
    kq_pass("q", 0, 0)
    # window-0 projection work, ordered by need-time (K fc_p gates pair p,
    # V chunk kc gates PV(kc) which recycles the es ring)
    fillers.extend([
        kf(1, 0), kf(2, 0), vf(0, 0), kf(3, 0), vf(0, 1), vf(0, 2), vf(0, 3),
        qf(0, 1), kf(0, 1), vf(1, 0), kf(1, 1), vf(1, 1), kf(2, 1), vf(1, 2),
        kf(3, 1), vf(1, 3), qf(0, 2), kf(0, 2), vf(2, 0), kf(1, 2), vf(2, 1),
        kf(2, 2), vf(2, 2), kf(3, 2), vf(2, 3), qf(0, 3), kf(0, 3), vf(3, 0),
        kf(1, 3), vf(3, 1), kf(2, 3), vf(3, 2), kf(3, 3), vf(3, 3),
    ])
    for i in (1, 2, 3):
        # bufs=1 ring: each DMA WAR-waits on the previous chunk's last
        # v_pass reader; V chunks are consumed strictly in order
        xv[i] = xpool.tile([P, DC, 512], BF16, name=f"xv{i}", tag="xv",
                           bufs=1)
        nc.sync.dma_start(xv[i][:], xsrc("xvT", i))
    # serialize all projection passes ahead of attention: LDWEIGHTS carries
    # no semaphore wait, so a stationary operand read too soon after its
    # DVE write reads stale SBUF (HW-verified); wide separation is safe
    while fillers:
        pop_filler()

    # --- main loop: q-windows of 512, head pairs, kc blocks of 2 ---
    for w in range(NW):
        if w + 1 < NW:
            # stage next window's xq chunk + its 4 projection passes
            xq[w + 1] = xpool.tile([P, DC, 512], BF16, name=f"xq{w + 1}",
                                   tag="xq", bufs=2)
            nc.sync.dma_start(xq[w + 1][:], xsrc("xqT", w + 1))
            for fc in range(NP):
                urgent.append((1700, (lambda ww, f: lambda: kq_pass(
                    "q", ww, f))(w + 1, fc)))
        for fc in range(NP):
            for kb in range(KC // 2):
                ensure_es(2)
                for kc in (2 * kb, 2 * kb + 1):
                    sps = ps_s.tile([P, 1024], F32, name=f"s{w}f{fc}k{kc}",
                                    tag="s")
                    nc.tensor.matmul(sps[:, 0:512],
                                     KT2[0:DH, fc, kc * P:(kc + 1) * P],
                                     QT[0:DH, fc, w * 512:(w + 1) * 512],
                                     start=True, stop=True)
                    nc.tensor.matmul(sps[:, 512:1024],
                                     KT2[DH:P, fc, kc * P:(kc + 1) * P],
                                     QT[DH:P, fc, w * 512:(w + 1) * 512],
                                     start=True, stop=True)
                    es = epool.tile([P, 1024], BF16, name=f"e{w}f{fc}k{kc}",
                                    tag="es")
                    nc.scalar.activation(es[:], sps[:], EXP, scale=SCALE)
                    est[(w, fc, kc)] = es
                    es_out[0] += 1
                pvq.append(make_pv(w, fc, 2 * kb))
                drain(1700)
            pvq.append(make_normalize(w, fc))
        for sc in range(4):
            for oc in range(2):
                pvq.append(make_outproj(w, sc, oc))
    while urgent or pvq or fillers:
        before = len(urgent) + len(pvq) + len(fillers)
        drain(float("inf"))
        after = len(urgent) + len(pvq) + len(fillers)
        assert after < before, "tail drain wedged"
    assert not (est or cps)


def build_program(num_devices=8):
    nc = bacc.Bacc("TRN2", target_bir_lowering=False, debug=False,
                   num_devices=num_devices)
    nqc = S // 512
    io = {
        "xqT": nc.dram_tensor("xqT", (nqc, P, DC * 512), BF16, kind="ExternalInput").ap(),
        "xkT": nc.dram_tensor("xkT", (nqc, P, DC * 512), BF16, kind="ExternalInput").ap(),
        "xvT": nc.dram_tensor("xvT", (nqc, P, DC * 512), BF16, kind="ExternalInput").ap(),
        "wqT": nc.dram_tensor("wqT", (P, DC, G), BF16, kind="ExternalInput").ap(),
        "wkT": nc.dram_tensor("wkT", (P, DC, G), BF16, kind="ExternalInput").ap(),
        "wvT": nc.dram_tensor("wvT", (P, DC, G), BF16, kind="ExternalInput").ap(),
        "woT": nc.dram_tensor("woT", (P, NP, D), BF16, kind="ExternalInput").ap(),
        "bqc": nc.dram_tensor("bqc", (G,), F32, kind="ExternalInput").ap(),
        "bkc": nc.dram_tensor("bkc", (G,), F32, kind="ExternalInput").ap(),
        "bvc": nc.dram_tensor("bvc", (G,), F32, kind="ExternalInput").ap(),
        "out": nc.dram_tensor("out", (S, D), F32, kind="ExternalOutput").ap(),
    }
    with tile.TileContext(nc) as tc:
        with ExitStack() as ctx:
            _emit(ctx, tc, io)
    nc.compile()
    return nc


_PROG = None


def _get_prog():
    global _PROG
    if _PROG is None:
        _PROG = build_program()
    return _PROG


def make_in_maps(q, k, v, wq, bq, wk, bk, wv, bv, wo):
    bf16 = ml_dtypes.bfloat16
    f32 = np.float32
    NQC = S // 512

    def xdev(x):
        # x: [S, D] -> [qc, p, dc*512] matching the SBUF chunk layout so each
        # DMA reads 8KB-contiguous per partition
        t = x.T.reshape(DC, P, NQC, 512).transpose(2, 1, 0, 3)
        return np.ascontiguousarray(t).astype(bf16).reshape(NQC, P, DC * 512)

    def wdev(w):
        # w rows slice: [G, D] -> wT [D, G] -> [p, dc, G]
        return np.ascontiguousarray(
            w.T.reshape(DC, P, G).transpose(1, 0, 2)).astype(bf16)

    xT = []
    for b in range(B):
        xT.append((xdev(q[b]), xdev(k[b]), xdev(v[b])))
    halves = []
    for hh in range(2):
        rows = slice(hh * G, (hh + 1) * G)
        halves.append({
            "wqT": wdev(wq[rows, :]),
            "wkT": wdev(wk[rows, :]),
            "wvT": wdev(wv[rows, :]),
            "woT": np.ascontiguousarray(
                wo[:, rows].T.reshape(NP, P, D).transpose(1, 0, 2)).astype(bf16),
            "bqc": np.ascontiguousarray(bq[rows]).astype(f32),
            "bkc": np.ascontiguousarray(bk[rows]).astype(f32),
            "bvc": np.ascontiguousarray(bv[rows]).astype(f32),
        })
    in_maps = []
    for c in range(8):
        b, hh = c // 2, c % 2
        m = dict(halves[hh])
        m["xqT"], m["xkT"], m["xvT"] = xT[b]
        in_maps.append(m)
    return in_maps


def run_with_results(q, k, v, wq, bq, wk, bk, wv, bv, wo, bo, **kw):
    nc = _get_prog()
    in_maps = make_in_maps(np.asarray(q, np.float32), np.asarray(k, np.float32),
                           np.asarray(v, np.float32), np.asarray(wq, np.float32),
                           np.asarray(bq, np.float32), np.asarray(wk, np.float32),
                           np.asarray(bk, np.float32), np.asarray(wv, np.float32),
                           np.asarray(bv, np.float32), np.asarray(wo, np.float32))
    res = run_bass_kernel_spmd(nc, in_maps, core_ids=list(range(8)), **kw)
    parts = [res.results[c]["out"] for c in range(8)]
    bo = np.asarray(bo, np.float32)
    out = np.stack([parts[2 * b] + parts[2 * b + 1] + bo for b in range(B)])
    return out.astype(np.float32), res


def kernel(q, k, v, wq, bq, wk, bk, wv, bv, wo, bo):
    out, _ = run_with_results(q, k, v, wq, bq, wk, bk, wv, bv, wo, bo)
    return out
